# revision 28
# baseline (speedup 1.0000x reference)
"""Trainium2 Bass kernel for KeypointSpatialAttention.

Math (per sample n):
    sampled[k, c] = bilinear_sample(feat[n], keypoint k)   -> S[n] @ feat[n].T
                    where S[n] is a (6, 49) sparse bilinear-weight matrix
                    (host-precomputed from kp_uv; invalid keypoints zeroed)
    h      = gelu(sampled @ W1 + b1)                        (6, 128)
    out[n] = (sum_k (h @ W2 + b2)) / n_valid                (256,)

Using associativity: sampled @ W1 = S @ (feat.T @ W1).  On device:
    stage 1: Z.T (128 HID part, 49 per n) = W1-chunk.T.T @ feat-chunk,
             accumulated over 8 C-chunks in PSUM.  float32r (full-rate
             fp32 matmul, moving dim >= 256), W1 stationary.
    transpose: per sample, PE-transpose Z.T (128, 49) -> Z (49, 128); pack
             two samples per PSUM tile at partition bases 0 and 64.
    stage 2: P.T-pair (128 HID, 12=2x6) = Z-pair.T.T @ ST-pair, where
             ST-pair (128, 12) has rows 0-48 = ST[n_even], 64-112 = ST[n_odd].
    gelu + bias b1 (per-partition, HID on partitions) on ScalarE
    reduce over k (innermost free axis) -> hsum.T (128 HID, n)
    stage 3: out-block (128 n, 256) = hsum.T.T @ W2  (+ ones.T @ 6*b2), then
             multiply by 1/n_valid (per-partition scalar), DMA out.

Sharding: pure data parallel over N=2048 across 8 cores (256 samples each).
"""

import ml_dtypes
import numpy as np

import concourse.bass as bass
from concourse import bacc
import concourse.mybir as mybir
import concourse.tile as tile
from concourse.bass_utils import run_bass_kernel_spmd

# Problem shapes (hardcoded; kernel.py must be self-contained).
N, C, FH, FW = 2048, 1024, 7, 7
NKP, HID, OUT = 6, 128, 256
N_CORES = 8
P = 128
HW49 = FH * FW          # 49
NS = N // N_CORES       # 256 samples per core
CH = C // P             # 8 contraction chunks
B = 32                  # samples per DMA group
G = NS // B             # 8 groups
HG = 4                  # half-groups per group (PSUM free-dim limit 512)
BH = B // HG            # 8 samples per half-group
NPAIR = NS // 2         # 128 pairs per core

F32 = mybir.dt.float32
F32R = mybir.dt.float32r
BF16 = mybir.dt.bfloat16

LAST_RESULTS = None
_NC_CACHE = {}


def _build_nc(reps=1):
    """Build the bass program.  reps>1 unrolls the whole pipeline body
    `reps` times over the same inputs/outputs inside one NEFF — used by the
    benchmark to measure the marginal (steady-state) per-execution HW time
    with per-dispatch overhead cancelled out."""
    nc = bacc.Bacc(trn_type="TRN2")

    feat_t = nc.dram_tensor("feat", (G, P, CH * B * HW49), BF16, kind="ExternalInput")
    w1_t = nc.dram_tensor("w1", (P, CH * HID), BF16, kind="ExternalInput")
    stp_t = nc.dram_tensor("stp", (2 * HW49, NPAIR * 2 * NKP), BF16, kind="ExternalInput")
    b1_t = nc.dram_tensor("b1", (P, 1), F32, kind="ExternalInput")
    w2_t = nc.dram_tensor("w2", (HID, OUT), F32, kind="ExternalInput")
    sixb2_t = nc.dram_tensor("sixb2", (1, OUT), F32, kind="ExternalInput")
    invnv_t = nc.dram_tensor("invnv", (P, NS // P), F32, kind="ExternalInput")
    ident_t = nc.dram_tensor("ident", (P, P), BF16, kind="ExternalInput")
    out_t = nc.dram_tensor("out", (NS, OUT), F32, kind="ExternalOutput")

    feat_ap = feat_t[:, :, :]
    out_ap = out_t[:, :]

    with tile.TileContext(nc) as tc:
        with (
            tc.tile_pool(name="const", bufs=1) as const,
            tc.tile_pool(name="featf", bufs=4) as featf,
            tc.tile_pool(name="ztsb", bufs=6) as ztsbp,
            tc.tile_pool(name="zpsb", bufs=8) as zpsbp,
            tc.tile_pool(name="outsb", bufs=2) as outp,
            tc.tile_pool(name="ztp", bufs=2, space="PSUM") as ztpp,
            tc.tile_pool(name="zpair", bufs=2, space="PSUM") as zpairp,
            tc.tile_pool(name="p2", bufs=2, space="PSUM") as p2p,
            tc.tile_pool(name="s3", bufs=1, space="PSUM") as s3pool,
        ):
            # ---- constants, loaded once ----
            w1t = const.tile([P, CH, HID], BF16)
            nc.sync.dma_start(w1t[:].rearrange("p c h -> p (c h)"), w1_t[:, :])
            stpt = const.tile([2 * HW49, NPAIR * 2 * NKP], BF16)
            nc.sync.dma_start(stpt[:], stp_t[:, :])
            b1t = const.tile([P, 1], F32)
            nc.sync.dma_start(b1t[:], b1_t[:, :])
            w2t = const.tile([HID, OUT], F32)
            nc.sync.dma_start(w2t[:], w2_t[:, :])
            sixb2t = const.tile([1, OUT], F32)
            nc.sync.dma_start(sixb2t[:], sixb2_t[:, :])
            invt = const.tile([P, NS // P], F32)
            nc.sync.dma_start(invt[:], invnv_t[:, :])
            identt = const.tile([P, P], BF16)
            nc.sync.dma_start(identt[:], ident_t[:, :])
            onest = const.tile([1, P], F32)
            nc.vector.memset(onest[:], 1.0)

            hTpre = const.tile([P, NS * NKP], F32)  # pre-gelu, (HID, n*k)
            hT = const.tile([P, NS * NKP], F32)     # gelu out, (HID, n*k)
            hsumT = const.tile([P, NS], F32)        # sum over k, (HID, n)

            # Observer ops: pre-absorb const-DMA sem waits so later PE
            # weight-load instructions carry at most one wait each (walrus
            # limit on the LDWEIGHTS word).
            dums = s3pool.tile([1, 8], F32)
            nc.tensor.matmul(dums[:, 0:2], w1t[:, 0, 0:1], w1t[:, 0, 0:2],
                             start=True, stop=True)
            nc.tensor.matmul(dums[:, 2:4], stpt[:, 0:1], stpt[:, 0:2],
                             start=True, stop=True)
            nc.tensor.matmul(dums[:, 4:6], identt[:, 0:1], identt[:, 0:2],
                             start=True, stop=True)
            scr_a = const.tile([P, 1], F32)
            nc.scalar.copy(out=scr_a[:], in_=b1t[:, 0:1])
            scr_v = const.tile([P, 1], F32)
            nc.vector.tensor_copy(out=scr_v[:], in_=invt[:, 0:1])

            for rep in range(reps):
              for g in range(G):
                # ---- one fused DMA: B samples x all 8 C-chunks, fully
                # contiguous per partition (128 descriptors of ~25 KB)
                ftg = featf.tile([P, CH, B * HW49], BF16, tag="featf")
                nc.sync.dma_start(
                    ftg[:].rearrange("p c f -> p (c f)"),
                    feat_ap[g, :, :],
                )

                p2 = p2p.tile([P, B * NKP], F32)
                for hg in range(HG):
                    # ---- stage 1: Z.T for BH samples, accum over C-chunks --
                    ztp = ztpp.tile([P, BH * HW49], F32)
                    for ch in range(CH):
                        nc.tensor.matmul(
                            ztp[:],
                            w1t[:, ch, :],
                            ftg[:, ch, hg * BH * HW49:(hg + 1) * BH * HW49],
                            start=(ch == 0),
                            stop=(ch == CH - 1),
                        )
                    ztsb = ztsbp.tile([P, BH * HW49], BF16, tag="ztsb")
                    nc.vector.tensor_copy(out=ztsb[:], in_=ztp[:])
                    # ---- transpose pairs + stage 2 ----
                    for pr in range(BH // 2):
                        zp = zpairp.tile([2 * HW49, HID], BF16)
                        nc.tensor.transpose(
                            zp[:, :],
                            ztsb[:, (2 * pr) * HW49:(2 * pr + 2) * HW49],
                            identt[:],
                        )
                        zpsb = zpsbp.tile([2 * HW49, HID], BF16, tag="zpsb")
                        nc.vector.tensor_copy(out=zpsb[:], in_=zp[:])
                        gq = g * (B // 2) + hg * (BH // 2) + pr
                        nc.tensor.matmul(
                            p2[:, (hg * (BH // 2) + pr) * 2 * NKP:
                               (hg * (BH // 2) + pr + 1) * 2 * NKP],
                            zpsb[:],
                            stpt[:, gq * 2 * NKP:(gq + 1) * 2 * NKP],
                            start=True,
                            stop=True,
                        )
                # ---- park pre-activations in SBUF (cheap DVE copy) ------
                nc.vector.tensor_copy(
                    out=hTpre[:, g * B * NKP:(g + 1) * B * NKP], in_=p2[:])

                # ---- once per 128-sample block (8 groups): one big gelu,
                # reduce over keypoints, stage 3, out DMA.  Block 0's tail
                # overlaps block 1's streaming.
                gpb = P // B  # groups per 128-sample block
                if (g + 1) % gpb == 0:
                    blk = g // gpb
                    nc.scalar.activation(
                        hT[:, blk * P * NKP:(blk + 1) * P * NKP],
                        hTpre[:, blk * P * NKP:(blk + 1) * P * NKP],
                        mybir.ActivationFunctionType.Gelu,
                        bias=b1t[:, 0:1],
                    )
                    nc.vector.reduce_sum(
                        hsumT[:, blk * P:(blk + 1) * P],
                        hT[:, blk * P * NKP:(blk + 1) * P * NKP].rearrange(
                            "p (n k) -> p n k", k=NKP
                        ),
                        axis=mybir.AxisListType.X,
                    )
                    s3 = s3pool.tile([P, OUT], F32)
                    nc.tensor.matmul(
                        s3[:], hsumT[:, blk * P:(blk + 1) * P], w2t[:],
                        start=True, stop=False,
                    )
                    nc.tensor.matmul(
                        s3[:], onest[:], sixb2t[:], start=False, stop=True,
                    )
                    osb = outp.tile([P, OUT], F32, tag="outsb")
                    nc.vector.tensor_scalar_mul(
                        osb[:], s3[:], invt[:, blk:blk + 1])
                    nc.sync.dma_start(
                        out_ap[blk * P:(blk + 1) * P, :], osb[:])

    nc.finalize()
    return nc


def _host_precompute(kp_uv, W1, b1, W2, b2,
                     crop_offset_x, crop_offset_y, crop_w, crop_h, img_w, img_h):
    """Replicate the reference coordinate transform in float32 and build the
    per-sample bilinear-weight matrices S (N, 6, 49), validity scaling, and
    the device-layout constant arrays."""
    f32 = np.float32
    kp = np.asarray(kp_uv, dtype=f32)
    u = kp[..., 0]
    v = kp[..., 1]
    px_x = u * f32(img_w)
    px_y = v * f32(img_h)
    crop_x = (px_x - f32(crop_offset_x)) / f32(crop_w)
    crop_y = (px_y - f32(crop_offset_y)) / f32(crop_h)
    grid_x = crop_x * f32(2.0) - f32(1.0)
    grid_y = crop_y * f32(2.0) - f32(1.0)

    invalid = (u < 0) | (v < 0)
    invalid |= (crop_x < 0) | (crop_x > 1) | (crop_y < 0) | (crop_y > 1)
    valid = (~invalid).astype(f32)                       # (N, NKP)

    ix = (grid_x + f32(1.0)) * f32(0.5) * f32(FW - 1)
    iy = (grid_y + f32(1.0)) * f32(0.5) * f32(FH - 1)
    x0 = np.floor(ix)
    y0 = np.floor(iy)
    x1 = x0 + f32(1.0)
    y1 = y0 + f32(1.0)
    wx1 = ix - x0
    wx0 = f32(1.0) - wx1
    wy1 = iy - y0
    wy0 = f32(1.0) - wy1

    S = np.zeros((N, NKP, HW49), dtype=f32)
    nn_idx, kk_idx = np.meshgrid(np.arange(N), np.arange(NKP), indexing="ij")
    for xi, yi, wgt in ((x0, y0, wx0 * wy0), (x1, y0, wx1 * wy0),
                        (x0, y1, wx0 * wy1), (x1, y1, wx1 * wy1)):
        inb = (xi >= 0) & (xi <= FW - 1) & (yi >= 0) & (yi <= FH - 1)
        xc = np.clip(xi, 0, FW - 1).astype(np.int64)
        yc = np.clip(yi, 0, FH - 1).astype(np.int64)
        idx = yc * FW + xc
        np.add.at(S, (nn_idx, kk_idx, idx), wgt * inb.astype(f32))
    S *= valid[:, :, None]

    n_valid = np.clip(valid.sum(axis=1), 1.0, None).astype(f32)   # (N,)
    invnv = (f32(1.0) / n_valid)

    # ST pairs with partition bases 0 / 64: (N/2, 128, 12)
    ST = np.transpose(S, (0, 2, 1))                       # (N, 49, 6)
    stp = np.zeros((N // 2, 2 * HW49, 2 * NKP), dtype=f32)
    stp[:, :HW49, :NKP] = ST[0::2]
    stp[:, HW49:, NKP:] = ST[1::2]

    W1 = np.asarray(W1, dtype=f32)
    w1_dev = np.ascontiguousarray(
        W1.reshape(CH, P, HID).transpose(1, 0, 2).reshape(P, CH * HID))
    b1_dev = np.ascontiguousarray(np.asarray(b1, dtype=f32).reshape(P, 1))
    w2_dev = np.ascontiguousarray(np.asarray(W2, dtype=f32).reshape(HID, OUT))
    sixb2_dev = (f32(NKP) * np.asarray(b2, dtype=f32)).reshape(1, OUT)
    return S, stp, invnv, w1_dev, b1_dev, w2_dev, sixb2_dev


def _make_in_maps(feat_map, kp_uv, W1, b1, W2, b2,
                  crop_offset_x, crop_offset_y, crop_w, crop_h, img_w, img_h):
    feat = np.ascontiguousarray(np.asarray(feat_map, dtype=np.float32))
    _, stp, invnv, w1_dev, b1_dev, w2_dev, sixb2_dev = _host_precompute(
        kp_uv, W1, b1, W2, b2,
        crop_offset_x, crop_offset_y, crop_w, crop_h, img_w, img_h)

    # (cores, G, P, CH, B, 49): per group g, partition p reads one fully
    # contiguous CH*B*49 run -> large DMA descriptors.  bf16 on device.
    featv = np.ascontiguousarray(
        feat.reshape(N_CORES, G, B, CH, P, HW49).transpose(0, 1, 4, 3, 2, 5)
    ).reshape(N_CORES, G, P, CH * B * HW49).astype(ml_dtypes.bfloat16)
    stpv = stp.reshape(N_CORES, NPAIR, 2 * HW49, 2 * NKP)
    invv = invnv.reshape(N_CORES, NS // P, P)
    ident = np.eye(P, dtype=ml_dtypes.bfloat16)

    in_maps = []
    for i in range(N_CORES):
        in_maps.append({
            "feat": featv[i],
            "w1": w1_dev.astype(ml_dtypes.bfloat16),
            "stp": np.ascontiguousarray(
                stpv[i].transpose(1, 0, 2)).reshape(
                    2 * HW49, NPAIR * 2 * NKP).astype(ml_dtypes.bfloat16),
            "b1": b1_dev,
            "w2": w2_dev,
            "sixb2": sixb2_dev,
            "invnv": np.ascontiguousarray(invv[i].T),
            "ident": ident,
        })
    return in_maps


def kernel(feat_map, kp_uv, W1, b1, W2, b2,
           crop_offset_x, crop_offset_y, crop_w, crop_h, img_w, img_h):
    global LAST_RESULTS
    in_maps = _make_in_maps(feat_map, kp_uv, W1, b1, W2, b2,
                            crop_offset_x, crop_offset_y, crop_w, crop_h,
                            img_w, img_h)
    if "nc" not in _NC_CACHE:
        _NC_CACHE["nc"] = _build_nc()
    nc = _NC_CACHE["nc"]

    res = run_bass_kernel_spmd(nc, in_maps, core_ids=list(range(N_CORES)))
    LAST_RESULTS = res
    out = np.concatenate([res.results[i]["out"] for i in range(N_CORES)], axis=0)
    return out.astype(np.float32)



# revision 32
# speedup vs baseline: 1.0571x; 1.0571x over previous
"""Trainium2 Bass kernel for KeypointSpatialAttention.

Math (per sample n):
    sampled[k, c] = bilinear_sample(feat[n], keypoint k)   -> S[n] @ feat[n].T
                    where S[n] is a (6, 49) sparse bilinear-weight matrix
                    (host-precomputed from kp_uv; invalid keypoints zeroed)
    h      = gelu(sampled @ W1 + b1)                        (6, 128)
    out[n] = (sum_k (h @ W2 + b2)) / n_valid                (256,)

Using associativity: sampled @ W1 = S @ (feat.T @ W1).  On device:
    stage 1: Z.T (128 HID part, 49 per n) = W1-chunk.T.T @ feat-chunk,
             accumulated over 8 C-chunks in PSUM.  float32r (full-rate
             fp32 matmul, moving dim >= 256), W1 stationary.
    transpose: per sample, PE-transpose Z.T (128, 49) -> Z (49, 128); pack
             two samples per PSUM tile at partition bases 0 and 64.
    stage 2: P.T-pair (128 HID, 12=2x6) = Z-pair.T.T @ ST-pair, where
             ST-pair (128, 12) has rows 0-48 = ST[n_even], 64-112 = ST[n_odd].
    gelu + bias b1 (per-partition, HID on partitions) on ScalarE
    reduce over k (innermost free axis) -> hsum.T (128 HID, n)
    stage 3: out-block (128 n, 256) = hsum.T.T @ W2  (+ ones.T @ 6*b2), then
             multiply by 1/n_valid (per-partition scalar), DMA out.

Sharding: pure data parallel over N=2048 across 8 cores (256 samples each).
"""

import ml_dtypes
import numpy as np

import concourse.bass as bass
from concourse import bacc
import concourse.mybir as mybir
import concourse.tile as tile
from concourse.bass_utils import run_bass_kernel_spmd

# Problem shapes (hardcoded; kernel.py must be self-contained).
N, C, FH, FW = 2048, 1024, 7, 7
NKP, HID, OUT = 6, 128, 256
N_CORES = 8
P = 128
HW49 = FH * FW          # 49
NS = N // N_CORES       # 256 samples per core
CH = C // P             # 8 contraction chunks
B = 16                  # samples per DMA group
G = NS // B             # 16 groups
HG = 2                  # half-groups per group (PSUM free-dim limit 512)
BH = B // HG            # 8 samples per half-group
NPAIR = NS // 2         # 128 pairs per core

F32 = mybir.dt.float32
F32R = mybir.dt.float32r
BF16 = mybir.dt.bfloat16

LAST_RESULTS = None
_NC_CACHE = {}


def _build_nc(reps=1):
    """Build the bass program.  reps>1 unrolls the whole pipeline body
    `reps` times over the same inputs/outputs inside one NEFF — used by the
    benchmark to measure the marginal (steady-state) per-execution HW time
    with per-dispatch overhead cancelled out."""
    nc = bacc.Bacc(trn_type="TRN2")

    feat_t = nc.dram_tensor("feat", (G, P, CH * B * HW49), BF16, kind="ExternalInput")
    w1_t = nc.dram_tensor("w1", (P, CH * HID), BF16, kind="ExternalInput")
    stp_t = nc.dram_tensor("stp", (2 * HW49, NPAIR * 2 * NKP), BF16, kind="ExternalInput")
    b1_t = nc.dram_tensor("b1", (P, 1), F32, kind="ExternalInput")
    w2_t = nc.dram_tensor("w2", (HID, OUT), F32, kind="ExternalInput")
    sixb2_t = nc.dram_tensor("sixb2", (1, OUT), F32, kind="ExternalInput")
    invnv_t = nc.dram_tensor("invnv", (P, NS // P), F32, kind="ExternalInput")
    ident_t = nc.dram_tensor("ident", (P, P), BF16, kind="ExternalInput")
    out_t = nc.dram_tensor("out", (NS, OUT), F32, kind="ExternalOutput")

    feat_ap = feat_t[:, :, :]
    out_ap = out_t[:, :]

    with tile.TileContext(nc) as tc:
        with (
            tc.tile_pool(name="const", bufs=1) as const,
            tc.tile_pool(name="featf", bufs=6) as featf,
            tc.tile_pool(name="ztsb", bufs=6) as ztsbp,
            tc.tile_pool(name="zpsb", bufs=8) as zpsbp,
            tc.tile_pool(name="outsb", bufs=2) as outp,
            tc.tile_pool(name="ztp", bufs=3, space="PSUM") as ztpp,
            tc.tile_pool(name="zpair", bufs=2, space="PSUM") as zpairp,
            tc.tile_pool(name="p2", bufs=2, space="PSUM") as p2p,
            tc.tile_pool(name="s3", bufs=1, space="PSUM") as s3pool,
        ):
            # ---- constants, loaded once ----
            w1t = const.tile([P, CH, HID], BF16)
            nc.sync.dma_start(w1t[:].rearrange("p c h -> p (c h)"), w1_t[:, :])
            stpt = const.tile([2 * HW49, NPAIR * 2 * NKP], BF16)
            nc.sync.dma_start(stpt[:], stp_t[:, :])
            b1t = const.tile([P, 1], F32)
            nc.sync.dma_start(b1t[:], b1_t[:, :])
            w2t = const.tile([HID, OUT], F32)
            nc.sync.dma_start(w2t[:], w2_t[:, :])
            sixb2t = const.tile([1, OUT], F32)
            nc.sync.dma_start(sixb2t[:], sixb2_t[:, :])
            invt = const.tile([P, NS // P], F32)
            nc.sync.dma_start(invt[:], invnv_t[:, :])
            identt = const.tile([P, P], BF16)
            nc.sync.dma_start(identt[:], ident_t[:, :])
            onest = const.tile([1, P], F32)
            nc.vector.memset(onest[:], 1.0)

            hTpre = const.tile([P, NS * NKP], F32)  # pre-gelu, (HID, n*k)
            hT = const.tile([P, NS * NKP], F32)     # gelu out, (HID, n*k)
            hsumT = const.tile([P, NS], F32)        # sum over k, (HID, n)

            # Observer ops: pre-absorb const-DMA sem waits so later PE
            # weight-load instructions carry at most one wait each (walrus
            # limit on the LDWEIGHTS word).
            dums = s3pool.tile([1, 8], F32, tag="s3")
            nc.tensor.matmul(dums[:, 0:2], w1t[:, 0, 0:1], w1t[:, 0, 0:2],
                             start=True, stop=True)
            nc.tensor.matmul(dums[:, 2:4], stpt[:, 0:1], stpt[:, 0:2],
                             start=True, stop=True)
            nc.tensor.matmul(dums[:, 4:6], identt[:, 0:1], identt[:, 0:2],
                             start=True, stop=True)
            scr_a = const.tile([P, 1], F32)
            nc.scalar.copy(out=scr_a[:], in_=b1t[:, 0:1])
            scr_v = const.tile([P, 1], F32)
            nc.vector.tensor_copy(out=scr_v[:], in_=invt[:, 0:1])

            for rep in range(reps):
              for g in range(G):
                # ---- one fused DMA: B samples x all 8 C-chunks, fully
                # contiguous per partition (128 descriptors of ~25 KB)
                ftg = featf.tile([P, CH, B * HW49], BF16, tag="featf")
                nc.sync.dma_start(
                    ftg[:].rearrange("p c f -> p (c f)"),
                    feat_ap[g, :, :],
                )

                p2 = p2p.tile([P, B * NKP], F32)
                for hg in range(HG):
                    # ---- stage 1: Z.T for BH samples, accum over C-chunks --
                    ztp = ztpp.tile([P, BH * HW49], F32)
                    for ch in range(CH):
                        nc.tensor.matmul(
                            ztp[:],
                            w1t[:, ch, :],
                            ftg[:, ch, hg * BH * HW49:(hg + 1) * BH * HW49],
                            start=(ch == 0),
                            stop=(ch == CH - 1),
                        )
                    ztsb = ztsbp.tile([P, BH * HW49], BF16, tag="ztsb")
                    nc.vector.tensor_copy(out=ztsb[:], in_=ztp[:])
                    # ---- transpose pairs + stage 2 ----
                    for pr in range(BH // 2):
                        zp = zpairp.tile([2 * HW49, HID], BF16)
                        nc.tensor.transpose(
                            zp[:, :],
                            ztsb[:, (2 * pr) * HW49:(2 * pr + 2) * HW49],
                            identt[:],
                        )
                        zpsb = zpsbp.tile([2 * HW49, HID], BF16, tag="zpsb")
                        nc.vector.tensor_copy(out=zpsb[:], in_=zp[:])
                        gq = g * (B // 2) + hg * (BH // 2) + pr
                        nc.tensor.matmul(
                            p2[:, (hg * (BH // 2) + pr) * 2 * NKP:
                               (hg * (BH // 2) + pr + 1) * 2 * NKP],
                            zpsb[:],
                            stpt[:, gq * 2 * NKP:(gq + 1) * 2 * NKP],
                            start=True,
                            stop=True,
                        )
                # ---- park pre-activations in SBUF (cheap DVE copy) ------
                nc.vector.tensor_copy(
                    out=hTpre[:, g * B * NKP:(g + 1) * B * NKP], in_=p2[:])

                # ---- once per 128-sample block (8 groups): one big gelu,
                # reduce over keypoints, stage 3, out DMA.  Block 0's tail
                # overlaps block 1's streaming.
                gpb = P // B  # groups per 128-sample block
                if (g + 1) % gpb == 0:
                    blk = g // gpb
                    nc.scalar.activation(
                        hT[:, blk * P * NKP:(blk + 1) * P * NKP],
                        hTpre[:, blk * P * NKP:(blk + 1) * P * NKP],
                        mybir.ActivationFunctionType.Gelu,
                        bias=b1t[:, 0:1],
                    )
                    nc.vector.reduce_sum(
                        hsumT[:, blk * P:(blk + 1) * P],
                        hT[:, blk * P * NKP:(blk + 1) * P * NKP].rearrange(
                            "p (n k) -> p n k", k=NKP
                        ),
                        axis=mybir.AxisListType.X,
                    )
                    s3 = s3pool.tile([P, OUT], F32)
                    nc.tensor.matmul(
                        s3[:], hsumT[:, blk * P:(blk + 1) * P], w2t[:],
                        start=True, stop=False,
                    )
                    nc.tensor.matmul(
                        s3[:], onest[:], sixb2t[:], start=False, stop=True,
                    )
                    osb = outp.tile([P, OUT], F32, tag="outsb")
                    nc.vector.tensor_scalar_mul(
                        osb[:], s3[:], invt[:, blk:blk + 1])
                    nc.sync.dma_start(
                        out_ap[blk * P:(blk + 1) * P, :], osb[:])

    nc.finalize()
    return nc


def _host_precompute(kp_uv, W1, b1, W2, b2,
                     crop_offset_x, crop_offset_y, crop_w, crop_h, img_w, img_h):
    """Replicate the reference coordinate transform in float32 and build the
    per-sample bilinear-weight matrices S (N, 6, 49), validity scaling, and
    the device-layout constant arrays."""
    f32 = np.float32
    kp = np.asarray(kp_uv, dtype=f32)
    u = kp[..., 0]
    v = kp[..., 1]
    px_x = u * f32(img_w)
    px_y = v * f32(img_h)
    crop_x = (px_x - f32(crop_offset_x)) / f32(crop_w)
    crop_y = (px_y - f32(crop_offset_y)) / f32(crop_h)
    grid_x = crop_x * f32(2.0) - f32(1.0)
    grid_y = crop_y * f32(2.0) - f32(1.0)

    invalid = (u < 0) | (v < 0)
    invalid |= (crop_x < 0) | (crop_x > 1) | (crop_y < 0) | (crop_y > 1)
    valid = (~invalid).astype(f32)                       # (N, NKP)

    ix = (grid_x + f32(1.0)) * f32(0.5) * f32(FW - 1)
    iy = (grid_y + f32(1.0)) * f32(0.5) * f32(FH - 1)
    x0 = np.floor(ix)
    y0 = np.floor(iy)
    x1 = x0 + f32(1.0)
    y1 = y0 + f32(1.0)
    wx1 = ix - x0
    wx0 = f32(1.0) - wx1
    wy1 = iy - y0
    wy0 = f32(1.0) - wy1

    S = np.zeros((N, NKP, HW49), dtype=f32)
    nn_idx, kk_idx = np.meshgrid(np.arange(N), np.arange(NKP), indexing="ij")
    for xi, yi, wgt in ((x0, y0, wx0 * wy0), (x1, y0, wx1 * wy0),
                        (x0, y1, wx0 * wy1), (x1, y1, wx1 * wy1)):
        inb = (xi >= 0) & (xi <= FW - 1) & (yi >= 0) & (yi <= FH - 1)
        xc = np.clip(xi, 0, FW - 1).astype(np.int64)
        yc = np.clip(yi, 0, FH - 1).astype(np.int64)
        idx = yc * FW + xc
        np.add.at(S, (nn_idx, kk_idx, idx), wgt * inb.astype(f32))
    S *= valid[:, :, None]

    n_valid = np.clip(valid.sum(axis=1), 1.0, None).astype(f32)   # (N,)
    invnv = (f32(1.0) / n_valid)

    # ST pairs with partition bases 0 / 64: (N/2, 128, 12)
    ST = np.transpose(S, (0, 2, 1))                       # (N, 49, 6)
    stp = np.zeros((N // 2, 2 * HW49, 2 * NKP), dtype=f32)
    stp[:, :HW49, :NKP] = ST[0::2]
    stp[:, HW49:, NKP:] = ST[1::2]

    W1 = np.asarray(W1, dtype=f32)
    w1_dev = np.ascontiguousarray(
        W1.reshape(CH, P, HID).transpose(1, 0, 2).reshape(P, CH * HID))
    b1_dev = np.ascontiguousarray(np.asarray(b1, dtype=f32).reshape(P, 1))
    w2_dev = np.ascontiguousarray(np.asarray(W2, dtype=f32).reshape(HID, OUT))
    sixb2_dev = (f32(NKP) * np.asarray(b2, dtype=f32)).reshape(1, OUT)
    return S, stp, invnv, w1_dev, b1_dev, w2_dev, sixb2_dev


def _make_in_maps(feat_map, kp_uv, W1, b1, W2, b2,
                  crop_offset_x, crop_offset_y, crop_w, crop_h, img_w, img_h):
    feat = np.ascontiguousarray(np.asarray(feat_map, dtype=np.float32))
    _, stp, invnv, w1_dev, b1_dev, w2_dev, sixb2_dev = _host_precompute(
        kp_uv, W1, b1, W2, b2,
        crop_offset_x, crop_offset_y, crop_w, crop_h, img_w, img_h)

    # (cores, G, P, CH, B, 49): per group g, partition p reads one fully
    # contiguous CH*B*49 run -> large DMA descriptors.  bf16 on device.
    featv = np.ascontiguousarray(
        feat.reshape(N_CORES, G, B, CH, P, HW49).transpose(0, 1, 4, 3, 2, 5)
    ).reshape(N_CORES, G, P, CH * B * HW49).astype(ml_dtypes.bfloat16)
    stpv = stp.reshape(N_CORES, NPAIR, 2 * HW49, 2 * NKP)
    invv = invnv.reshape(N_CORES, NS // P, P)
    ident = np.eye(P, dtype=ml_dtypes.bfloat16)

    in_maps = []
    for i in range(N_CORES):
        in_maps.append({
            "feat": featv[i],
            "w1": w1_dev.astype(ml_dtypes.bfloat16),
            "stp": np.ascontiguousarray(
                stpv[i].transpose(1, 0, 2)).reshape(
                    2 * HW49, NPAIR * 2 * NKP).astype(ml_dtypes.bfloat16),
            "b1": b1_dev,
            "w2": w2_dev,
            "sixb2": sixb2_dev,
            "invnv": np.ascontiguousarray(invv[i].T),
            "ident": ident,
        })
    return in_maps


def kernel(feat_map, kp_uv, W1, b1, W2, b2,
           crop_offset_x, crop_offset_y, crop_w, crop_h, img_w, img_h):
    global LAST_RESULTS
    in_maps = _make_in_maps(feat_map, kp_uv, W1, b1, W2, b2,
                            crop_offset_x, crop_offset_y, crop_w, crop_h,
                            img_w, img_h)
    if "nc" not in _NC_CACHE:
        _NC_CACHE["nc"] = _build_nc()
    nc = _NC_CACHE["nc"]

    res = run_bass_kernel_spmd(nc, in_maps, core_ids=list(range(N_CORES)))
    LAST_RESULTS = res
    out = np.concatenate([res.results[i]["out"] for i in range(N_CORES)], axis=0)
    return out.astype(np.float32)

